# revision 11
# baseline (speedup 1.0000x reference)
# BitLinear (eval path) Trainium2 kernel: ternary weight quant + int8-grade
# activation quant + dense matmul, tensor-parallel over 8 NeuronCores.
#
# Math (per reference):
#   w_scale[o] = max(mean_k |W[o,k]|, EPS)
#   w_quant    = clip(round(W / w_scale), -1, 1)            (ternary, exact)
#   x_scale[t] = max(max_k |x[t,k]| / 127, EPS)
#   xq[t,k]    = bf16(x[t,k] / x_scale[t])   -- bf16 grid is finer than the
#                int8 grid below 128, so skipping the integer round costs
#                ~9e-3 rel err vs the int8 reference (budget 2e-2).
#   out[t,o]   = (sum_k xq[t,k] * w_quant[o,k]) * x_scale[t] * w_scale[o] + bias[o]
#   (outputs stored bf16, upcast on host: +~2e-3 in quadrature)
#
# W ternary stays exact: magic-constant rounding (v + 1.5*2^23) on the scalar
# engine; the ternary clip is fused into the post-transpose fp8 cast on DVE.
#
# The frontend (load -> reduce -> quantize -> xbar-transpose) is a latency-
# bound pipeline; with full [128,4096] tiles only 2 fit per pool and block
# latency ~60us paced everything. All frontend stages run at HALF-K
# granularity ([128,2048] tiles, 4 bufs) so 4 units are in flight per chain
# and the DMA queues stay fed.
#
# Queue split: sync HWDGE = x loads + all transposes; scalar HWDGE = W loads
# + output stores; gpsimd = bias fetch + epilogue bias add; DVE = reduces,
# scale chains, fused clip+fp8, epilogue scale; PE = 2048 bf16 matmuls
# (437us roofline = the target bottleneck).
import numpy as np

import concourse.bacc as bacc
import concourse.bass as bass
import concourse.tile as tile
from concourse import mybir
from concourse.bass_utils import run_bass_kernel_spmd
from concourse.masks import make_identity

F32 = mybir.dt.float32
BF16 = mybir.dt.bfloat16
FP8 = mybir.dt.float8e4

EPS = 1e-5
MAGIC = 12582912.0  # 1.5 * 2^23

B, S, I, O = 4, 2048, 4096, 4096
T_FULL = B * S
TSPLIT, OSPLIT = 4, 2
N_CORES = TSPLIT * OSPLIT

A = mybir.AluOpType
AF = mybir.ActivationFunctionType


def build_nc(K=I, TO=O // OSPLIT, TT=T_FULL // TSPLIT):
    """Per-core program: x [TT, K], w [TO, K], bias [TO] -> out [TT, TO]."""
    KT = K // 128      # 32 k subtiles
    KH = K // 2        # 2048: half-K frontend granularity
    KTH = KT // 2      # 16 k subtiles per half
    GT = 128           # tokens / out-rows per group
    NG = TT // GT      # 16 token groups
    NB = TO // GT      # 16 W blocks
    OC = 512           # moving width per matmul
    NOC = TO // OC     # 4 o-chunks

    nc = bacc.Bacc("TRN2", target_bir_lowering=False, debug=False)
    x_d = nc.dram_tensor("x", [TT, K], F32, kind="ExternalInput").ap()
    w_d = nc.dram_tensor("w", [TO, K], F32, kind="ExternalInput").ap()
    bias_d = nc.dram_tensor("bias", [TO], F32, kind="ExternalInput").ap()
    out_d = nc.dram_tensor("out", [TT, TO], BF16, kind="ExternalOutput").ap()

    with tile.TileContext(nc) as tc:
        with (
            tc.tile_pool(name="lx", bufs=4) as p_lx,      # f32 x half-row loads
            tc.tile_pool(name="lw", bufs=4) as p_lw,      # f32 W half-row loads
            tc.tile_pool(name="btx", bufs=2) as p_btx,    # bf16 quantized x halves
            tc.tile_pool(name="btw", bufs=2) as p_btw,    # bf16 quantized W halves
            tc.tile_pool(name="wst", bufs=2) as p_wst,    # transposed W staging
            tc.tile_pool(name="wq", bufs=1) as p_wq,      # resident fp8 weights
            tc.tile_pool(name="xq", bufs=4) as p_xq,      # bf16 K-major token tiles
            tc.tile_pool(name="sml", bufs=6) as p_sml,
            tc.tile_pool(name="osb", bufs=2) as p_osb,
            tc.tile_pool(name="const", bufs=1) as p_const,
            tc.tile_pool(name="ps_mm", bufs=5, space="PSUM") as ps_mm,
            tc.tile_pool(name="ps_tr", bufs=1, space="PSUM") as ps_tr,
        ):
            ident = p_const.tile([128, 128], F32)
            make_identity(nc, ident[:])
            ones_row = p_const.tile([1, 128], BF16)
            nc.vector.memset(ones_row[:], 1.0)
            mag_col = p_const.tile([128, 1], F32)
            nc.vector.memset(mag_col[:], MAGIC)
            nmag_col = p_const.tile([128, 1], F32)
            nc.vector.memset(nmag_col[:], -MAGIC)
            xs_cols = p_const.tile([128, NG], F32)    # x_scale, t on partitions
            ws_epi = p_const.tile([128, TO], BF16)    # w_scale bcast rows
            bias_bc = p_const.tile([128, TO], BF16)   # bias bcast rows

            wq_oc = [
                p_wq.tile([128, KT, OC], FP8, name=f"wq_{oc}") for oc in range(NOC)
            ]
            xq_tiles = {}
            lx_tiles = {}
            lw_tiles = {}

            # ---------- loads: half-K tiles, each flow on its own queue ------
            def x_load(tg):
                for h in range(2):
                    xg = p_lx.tile([128, KH], F32, tag="lx")
                    nc.sync.dma_start(
                        out=xg[:],
                        in_=x_d[tg * GT : (tg + 1) * GT, h * KH : (h + 1) * KH],
                    )
                    lx_tiles[(tg, h)] = xg

            def w_load(ob):
                for h in range(2):
                    wg = p_lw.tile([128, KH], F32, tag="lw")
                    nc.scalar.dma_start(
                        out=wg[:],
                        in_=w_d[ob * GT : (ob + 1) * GT, h * KH : (h + 1) * KH],
                    )
                    lw_tiles[(ob, h)] = wg

            # ---------- x group: amax, one-pass quantize, xbar transpose ----
            def x_quant(tg):
                xga = lx_tiles.pop((tg, 0))
                xgb = lx_tiles.pop((tg, 1))
                ama = p_sml.tile([128, 1], F32, tag="am")
                amb = p_sml.tile([128, 1], F32, tag="am")
                nc.vector.tensor_reduce(
                    out=ama[:], in_=xga[:], axis=mybir.AxisListType.X,
                    op=A.max, apply_absolute_value=True,
                )
                nc.vector.tensor_reduce(
                    out=amb[:], in_=xgb[:], axis=mybir.AxisListType.X,
                    op=A.max, apply_absolute_value=True,
                )
                am = p_sml.tile([128, 1], F32, tag="am")
                nc.vector.tensor_tensor(out=am[:], in0=ama[:], in1=amb[:], op=A.max)
                nc.vector.tensor_scalar(
                    out=xs_cols[:, tg : tg + 1], in0=am[:],
                    scalar1=1.0 / 127.0, scalar2=EPS, op0=A.mult, op1=A.max,
                )
                rxs = p_sml.tile([128, 1], F32, tag="rxs")
                nc.vector.reciprocal(rxs[:], xs_cols[:, tg : tg + 1])
                xq_t = p_xq.tile([128, KT, GT], BF16, tag="xq")
                for h, xg in ((0, xga), (1, xgb)):
                    xot = p_btx.tile([128, KH], BF16, tag="btx")
                    nc.scalar.activation(
                        out=xot[:], in_=xg[:], func=AF.Identity, scale=rxs[:],
                    )
                    nc.sync.dma_start_transpose(
                        xq_t[:, h * KTH : (h + 1) * KTH, :], xot[:]
                    )
                xq_tiles[tg] = xq_t

            # ---------- W block: scales, ternary, transpose, clip+fp8 -------
            def w_quant(ob):
                wga = lw_tiles.pop((ob, 0))
                wgb = lw_tiles.pop((ob, 1))
                wsa = p_sml.tile([128, 1], F32, tag="ws")
                wsb = p_sml.tile([128, 1], F32, tag="ws")
                nc.vector.tensor_reduce(
                    out=wsa[:], in_=wga[:], axis=mybir.AxisListType.X,
                    op=A.add, apply_absolute_value=True,
                )
                nc.vector.tensor_reduce(
                    out=wsb[:], in_=wgb[:], axis=mybir.AxisListType.X,
                    op=A.add, apply_absolute_value=True,
                )
                wsum = p_sml.tile([128, 1], F32, tag="ws")
                nc.vector.tensor_tensor(out=wsum[:], in0=wsa[:], in1=wsb[:], op=A.add)
                wsf = p_sml.tile([128, 1], F32, tag="ws")
                nc.vector.tensor_scalar(
                    out=wsf[:], in0=wsum[:], scalar1=1.0 / K, scalar2=EPS,
                    op0=A.mult, op1=A.max,
                )
                rws = p_sml.tile([128, 1], F32, tag="ws")
                nc.vector.reciprocal(rws[:], wsf[:])
                # ws column -> [1,128] row (PE transpose) -> DVE copy to SBUF
                # -> K=1 ones matmul broadcasts to 128 partitions -> ws_epi
                ptr = ps_tr.tile([1, 128], F32, tag="tr")
                nc.tensor.transpose(ptr[:], wsf[:], ident[:])
                wsrow = p_sml.tile([1, 128], BF16, tag="wsrow")
                nc.vector.tensor_copy(wsrow[:], ptr[:])
                pbc = ps_tr.tile([128, 128], F32, tag="bc")
                nc.tensor.matmul(pbc[:], ones_row[:], wsrow[:], start=True, stop=True)
                nc.vector.tensor_copy(ws_epi[:, ob * GT : (ob + 1) * GT], pbc[:])
                # u = w*(1/ws) + M (exact rint in the f32 add), then u - M in
                # bf16 (small ints, exact); ternary clip fused into fp8 cast
                oc, osl = ob // (OC // GT), (ob % (OC // GT)) * GT
                for h, wg in ((0, wga), (1, wgb)):
                    nc.scalar.activation(
                        out=wg[:], in_=wg[:], func=AF.Identity,
                        scale=rws[:], bias=mag_col[:],
                    )
                    wot = p_btw.tile([128, KH], BF16, tag="btw")
                    nc.scalar.activation(
                        out=wot[:], in_=wg[:], func=AF.Identity, bias=nmag_col[:],
                    )
                    wstg = p_wst.tile([128, KTH, GT], BF16, tag="wst")
                    nc.sync.dma_start_transpose(wstg[:], wot[:])
                    nc.vector.tensor_scalar(
                        out=wq_oc[oc][:, h * KTH : (h + 1) * KTH, osl : osl + GT],
                        in0=wstg[:],
                        scalar1=1.0, scalar2=-1.0, op0=A.min, op1=A.max,
                    )

            # ---------- matmul pass ----------
            def mm_pass(tg, oc):
                xq_t = xq_tiles[tg]
                pm = ps_mm.tile([128, OC], F32, tag="mm")
                for kt in range(KT):
                    nc.tensor.matmul(
                        pm[:],
                        xq_t[:, kt, :],
                        wq_oc[oc][:, kt, :],
                        start=(kt == 0),
                        stop=(kt == KT - 1),
                    )
                osb = p_osb.tile([128, OC], BF16, tag="osb")
                nc.vector.scalar_tensor_tensor(
                    out=osb[:], in0=pm[:], scalar=xs_cols[:, tg : tg + 1],
                    in1=ws_epi[:, oc * OC : (oc + 1) * OC], op0=A.mult, op1=A.mult,
                )
                nc.gpsimd.tensor_tensor(
                    out=osb[:], in0=osb[:],
                    in1=bias_bc[:, oc * OC : (oc + 1) * OC], op=A.add,
                )
                nc.scalar.dma_start(
                    out=out_d[tg * GT : (tg + 1) * GT, oc * OC : (oc + 1) * OC],
                    in_=osb[:],
                )

            # ---------- main schedule ----------
            # Loads kick off first on their own queues; pools pace them.
            x_load(0)
            w_load(0)
            x_load(1)
            w_load(1)
            nc.gpsimd.dma_start(
                out=bias_bc[:],
                in_=bass.AP(
                    tensor=bias_d.tensor, offset=bias_d.offset,
                    ap=[[0, 128], [1, TO]],
                ),
            )
            x_quant(0)
            w_quant(0)
            x_load(2)
            w_load(2)
            w_quant(1)
            x_load(3)
            w_load(3)
            w_quant(2)
            x_quant(1)
            w_load(4)
            w_load(5)
            w_quant(3)
            # Section 0, oc-major; W blocks 4-15 stream through the mm shadow.
            mm_pass(0, 0)
            x_quant(2)
            w_quant(4)
            mm_pass(1, 0)
            x_quant(3)
            w_load(6)
            w_load(7)
            w_quant(5)
            mm_pass(2, 0)
            w_quant(6)
            mm_pass(3, 0)
            w_load(8)
            w_load(9)
            w_quant(7)
            mm_pass(0, 1)
            w_quant(8)
            mm_pass(1, 1)
            w_load(10)
            w_load(11)
            w_quant(9)
            mm_pass(2, 1)
            w_quant(10)
            mm_pass(3, 1)
            w_load(12)
            w_load(13)
            w_quant(11)
            mm_pass(0, 2)
            w_quant(12)
            mm_pass(1, 2)
            w_load(14)
            w_load(15)
            w_quant(13)
            mm_pass(2, 2)
            w_quant(14)
            mm_pass(3, 2)
            w_quant(15)
            # oc3 of section 0 frees xq tiles -> prefetch next section's x.
            mm_pass(0, 3)
            x_load(4)
            x_quant(4)
            mm_pass(1, 3)
            x_load(5)
            x_quant(5)
            mm_pass(2, 3)
            x_load(6)
            x_quant(6)
            mm_pass(3, 3)
            x_load(7)
            x_quant(7)
            # Sections 1-3: tg-major (all W resident); prefetch x as tiles free.
            for sec in range(1, 4):
                for tg in range(sec * 4, sec * 4 + 4):
                    for oc in range(NOC):
                        mm_pass(tg, oc)
                    nxt = tg + 4
                    if nxt < NG:
                        x_load(nxt)
                        x_quant(nxt)
    nc.compile()
    return nc


_NC_CACHE = {}
LAST_EXEC_NS = None


def _get_nc():
    if "full" not in _NC_CACHE:
        _NC_CACHE["full"] = build_nc()
    return _NC_CACHE["full"]


def _run(x, weight, bias, trace=False):
    global LAST_EXEC_NS
    x = np.asarray(x, dtype=np.float32).reshape(T_FULL, I)
    weight = np.asarray(weight, dtype=np.float32)
    bias = np.asarray(bias, dtype=np.float32)

    TT = T_FULL // TSPLIT
    TO = O // OSPLIT
    in_maps = []
    for c in range(N_CORES):
        ti, oj = divmod(c, OSPLIT)
        in_maps.append(
            {
                "x": np.ascontiguousarray(x[ti * TT : (ti + 1) * TT, :]),
                "w": np.ascontiguousarray(weight[oj * TO : (oj + 1) * TO, :]),
                "bias": np.ascontiguousarray(bias[oj * TO : (oj + 1) * TO]),
            }
        )

    nc = _get_nc()
    res = run_bass_kernel_spmd(
        nc, in_maps, core_ids=list(range(N_CORES)), trace=trace
    )
    LAST_EXEC_NS = res.exec_time_ns

    out = np.empty((T_FULL, O), dtype=np.float32)
    for c in range(N_CORES):
        ti, oj = divmod(c, OSPLIT)
        out[ti * TT : (ti + 1) * TT, oj * TO : (oj + 1) * TO] = np.asarray(
            res.results[c]["out"]
        ).astype(np.float32)
    return out.reshape(B, S, O)


def kernel(x, weight, bias):
    return _run(x, weight, bias, trace=False)


def kernel_traced(x, weight, bias):
    _run(x, weight, bias, trace=True)
    return LAST_EXEC_NS
